# revision 12
# baseline (speedup 1.0000x reference)
"""Trainium2 Bass kernel for nn_BoundaryLoss (8-core SPMD).

Strategy
--------
Shard the label axis (K=150, padded to 152 = 8*19) across the 8 cores.
Per core, for each of its 19 labels k:
    dist2[b, :] accumulation:  psum[b_chunk, i] += oodT[j, b_chunk].T @ R_k.T[j, i]
  (out layout [b, i] so the norm reduction is a free-axis reduce), then a
  fused Square+accumulate (ScalarE) produces dist^2 per (label, b_chunk).
The positive term rides along as a 5th "b_chunk" whose stationary operand is
the core's own gathered (pooled - centroid[label]) columns; a host-built mask
selects the (slot, label) pairs that are real.
A tiny batched epilogue (sqrt/exp/relu/sign + masked reduces) turns dist^2
into the four partial sums [pos_loss_sum, neg_loss_sum(masked per-label),
pos_num, neg_num] per partition; the host sums partitions and cores and forms
the 5 scalar outputs.

Matmuls run as float32r (full-rate fp32) with N=512/256 moving tiles.
"""

import math

import numpy as np

try:
    import concourse.bacc as bacc
    import concourse.mybir as mybir
    import concourse.tile as tile
    from concourse.bass_utils import run_bass_kernel_spmd
except ImportError:  # pragma: no cover - fallback for bare environments
    import sys

    sys.path.insert(0, "/opt/trn_rl_repo")
    import concourse.bacc as bacc
    import concourse.mybir as mybir
    import concourse.tile as tile
    from concourse.bass_utils import run_bass_kernel_spmd

K = 150
D = 768
B = 512
BETA = 0.3
NCORES = 8
KPC = 19                # labels per core
KPAD = NCORES * KPC     # 152
NJ = D // 128           # 6 contraction chunks
NBC = 5                 # 4 ood b-chunks + 1 pos chunk
NCOL = KPC * NBC        # 95 accumulator columns per core
F32 = mybir.dt.float32

_prog_cache = {}


def build_program(mode="f32r", debug_acc=False, warmup=16, tag=None,
                  rt_split_labels=KPC, rt_bufs=4):
    """Build the SPMD bass program. mode in {"f32r", "f32", "bf16"}."""
    store_dt = {"f32r": mybir.dt.float32r, "f32": F32,
                "bf16": mybir.dt.bfloat16}[mode]
    nc = bacc.Bacc("TRN2", target_bir_lowering=False, debug=False,
                   num_devices=NCORES)
    if tag is not None:
        nc.dram_tensor(f"tag_{tag}", [1, 1], F32, kind="ExternalInput")
    accdump = None
    if debug_acc:
        accdump = nc.dram_tensor("accdump", [128, NCOL], F32,
                                 kind="ExternalOutput").ap()

    rt = nc.dram_tensor("rt", [KPC, 128, NJ * D], store_dt,
                        kind="ExternalInput").ap()
    oodt = nc.dram_tensor("oodt", [128, NJ * B], store_dt,
                          kind="ExternalInput").ap()
    ck = nc.dram_tensor("ck", [128, KPC * NJ], F32,
                        kind="ExternalInput").ap()
    xpost = nc.dram_tensor("xpost", [128, NJ * 128], store_dt,
                           kind="ExternalInput").ap()
    dk = nc.dram_tensor("dk", [128, NCOL], F32, kind="ExternalInput").ap()
    oodmask = nc.dram_tensor("oodmask", [128, NCOL], F32,
                             kind="ExternalInput").ap()
    posmask = nc.dram_tensor("posmask", [128, NCOL], F32,
                             kind="ExternalInput").ap()
    partials = nc.dram_tensor("partials", [128, 4], F32,
                              kind="ExternalOutput").ap()

    def mmcast(ap):
        return ap

    # Register the Exp bias constant (ln BETA) the way Bass.__init__ does.
    LNBETA = float(math.log(BETA))
    const_t = nc.alloc_sbuf_tensor("const-lnbeta", [128, 1], F32)
    nc.gpsimd.memset(const_t.ap(), LNBETA)
    nc.const_aps.aps[(F32, LNBETA)] = const_t.ap()
    nc.all_engine_barrier()

    AF = mybir.ActivationFunctionType
    ALU = mybir.AluOpType

    with tile.TileContext(nc) as tc:
        with (
            tc.tile_pool(name="consts", bufs=1) as cpool,
            tc.tile_pool(name="rtp", bufs=rt_bufs) as rtpool,
            tc.tile_pool(name="scratch", bufs=3) as spool,
            tc.tile_pool(name="psum", bufs=4, space="PSUM") as ppool,
            tc.tile_pool(name="fin", bufs=1) as fpool,
        ):
            # Load order matters for the critical path: oodt+ck feed the
            # first oodc subtraction; xpost is first needed at bc==4 of
            # label 0; dk/masks only at the final epilogue.
            ck_s = cpool.tile([128, KPC * NJ], F32)
            nc.sync.dma_start(out=ck_s[:], in_=ck[:])
            oodt_s = cpool.tile([128, NJ * B], store_dt)
            for jc in range(NJ):
                nc.sync.dma_start(out=oodt_s[:, jc * B:(jc + 1) * B],
                                  in_=oodt[:, jc * B:(jc + 1) * B])
            xpost_s = cpool.tile([128, NJ * 128], store_dt)
            nc.sync.dma_start(out=xpost_s[:], in_=xpost[:])
            dk_s = cpool.tile([128, NCOL], F32)
            om_s = cpool.tile([128, NCOL], F32)
            pm_s = cpool.tile([128, NCOL], F32)
            acc = cpool.tile([128, NCOL], F32)

            # Optional HAM warm-up: dummy matmuls on the (already loaded)
            # oodt tile while the first rt DMA is in flight. These have no
            # data deps on rt, so Tile schedules them first; they warm the
            # PE clock gate so the real stream runs at 2.4 GHz sooner.
            for w in range(warmup):
                wps = ppool.tile([128, 1024], F32, tag="ps")
                nc.tensor.matmul(wps[:, 0:512],
                                 oodt_s[:, 0:128], oodt_s[:, 0:512],
                                 start=True, stop=True)

            for k in range(KPC):
                rtk = rtpool.tile([128, NJ * D], store_dt)
                if k < rt_split_labels:
                    # split the load so this label's first matmuls can
                    # start after ~1/6 of the transfer
                    for jc in range(NJ):
                        nc.sync.dma_start(
                            out=rtk[:, jc * D:(jc + 1) * D],
                            in_=rt[k, :, jc * D:(jc + 1) * D])
                else:
                    nc.sync.dma_start(out=rtk[:], in_=rt[k, :, :])
                # oodc = oodT - c_k (c_k is a per-partition scalar per jc)
                oodc = spool.tile([128, NJ * B], store_dt, tag="oodc")
                for jc in range(NJ):
                    nc.vector.tensor_scalar_sub(
                        oodc[:, jc * B:(jc + 1) * B],
                        oodt_s[:, jc * B:(jc + 1) * B],
                        ck_s[:, k * NJ + jc:k * NJ + jc + 1])
                for bc in range(NBC):
                    ps = ppool.tile([128, 1024], F32, tag="ps")
                    for jc in range(NJ):
                        if bc < 4:
                            lhsT = oodc[:, jc * B + bc * 128:
                                        jc * B + (bc + 1) * 128]
                        else:
                            lhsT = xpost_s[:, jc * 128:(jc + 1) * 128]
                        nc.tensor.matmul(
                            ps[:, 0:512], mmcast(lhsT),
                            mmcast(rtk[:, jc * D:jc * D + 512]),
                            start=(jc == 0), stop=(jc == NJ - 1))
                        nc.tensor.matmul(
                            ps[:, 512:768], mmcast(lhsT),
                            mmcast(rtk[:, jc * D + 512:(jc + 1) * D]),
                            start=(jc == 0), stop=(jc == NJ - 1))
                    sq = spool.tile([128, D], F32)
                    col = k * NBC + bc
                    nc.scalar.activation(sq[:], ps[:, 0:768], AF.Square,
                                         accum_out=acc[:, col:col + 1])

            nc.sync.dma_start(out=dk_s[:], in_=dk[:])
            nc.sync.dma_start(out=om_s[:], in_=oodmask[:])
            nc.sync.dma_start(out=pm_s[:], in_=posmask[:])

            if debug_acc:
                nc.sync.dma_start(out=accdump[:], in_=acc[:])
            # ---- batched epilogue over the [128, 95] accumulator ----
            dist = fpool.tile([128, NCOL], F32)
            nc.scalar.activation(dist[:], acc[:], AF.Sqrt)
            t = fpool.tile([128, NCOL], F32)        # t = dist - dk
            nc.vector.tensor_tensor(out=t[:], in0=dist[:], in1=dk_s[:],
                                    op=ALU.subtract)
            relu_t = fpool.tile([128, NCOL], F32)   # (euc - d)+
            nc.scalar.activation(relu_t[:], t[:], AF.Relu)
            nrelu = fpool.tile([128, NCOL], F32)    # (d - euc)+
            nc.scalar.activation(nrelu[:], t[:], AF.Relu, scale=-1.0)
            m = fpool.tile([128, NCOL], F32)        # 1[d > euc]
            nc.scalar.activation(m[:], nrelu[:], AF.Sign)
            pnum_i = fpool.tile([128, NCOL], F32)   # 1[euc > d]
            nc.scalar.activation(pnum_i[:], relu_t[:], AF.Sign)
            e = fpool.tile([128, NCOL], F32)        # beta * exp(dk - dist)
            nc.scalar.activation(e[:], t[:], AF.Exp, scale=-1.0,
                                 bias=LNBETA)
            # e is only selected when dk <= dist, where e <= beta; clamp so
            # the branchless blend below can't catastrophically cancel.
            nc.vector.tensor_scalar_min(e[:], e[:], BETA)
            inb = fpool.tile([128, NCOL], F32)      # dk - dist + beta
            nc.scalar.activation(inb[:], t[:], AF.Copy, scale=-1.0,
                                 bias=BETA)
            # pl = e + m * (inb - e)   (branchless where(dk > dist, inb, e))
            d1 = fpool.tile([128, NCOL], F32)
            nc.vector.tensor_tensor(out=d1[:], in0=inb[:], in1=e[:],
                                    op=ALU.subtract)
            d2 = fpool.tile([128, NCOL], F32)
            nc.vector.tensor_tensor(out=d2[:], in0=m[:], in1=d1[:],
                                    op=ALU.mult)
            pl = fpool.tile([128, NCOL], F32)
            nc.vector.tensor_tensor(out=pl[:], in0=e[:], in1=d2[:],
                                    op=ALU.add)

            out4 = fpool.tile([128, 4], F32)
            for idx, (a, b) in enumerate([(pl, om_s), (relu_t, pm_s),
                                          (pnum_i, pm_s), (m, pm_s)]):
                tmp = fpool.tile([128, NCOL], F32, tag="redtmp")
                nc.vector.tensor_tensor(out=tmp[:], in0=a[:], in1=b[:],
                                        op=ALU.mult)
                nc.vector.tensor_reduce(out=out4[:, idx:idx + 1], in_=tmp[:],
                                        axis=mybir.AxisListType.X, op=ALU.add)
            nc.sync.dma_start(out=partials[:], in_=out4[:])

    nc.compile()
    return nc


def prep_inputs(pooled_output, centroids, delta, L, U, D_diag, ood, labels,
                mode="f32r"):
    """Host-side shard prep. Returns in_maps (list of 8 dicts)."""
    pooled_output = np.asarray(pooled_output, np.float32)
    centroids = np.asarray(centroids, np.float32)
    delta = np.asarray(delta, np.float32)
    L = np.asarray(L, np.float32)
    U = np.asarray(U, np.float32)
    D_diag = np.asarray(D_diag, np.float32)
    ood = np.asarray(ood, np.float32)
    labels = np.asarray(labels).astype(np.int64)

    store_np = np.float32
    if mode == "bf16":
        import ml_dtypes
        store_np = ml_dtypes.bfloat16

    # RT[k] = R[k].T built directly in the DMA layout [k, p, jc*768 + i]:
    # element (row a, col b) of R.T lives at partition a%128, free (a//128)*768+b.
    rows, cols = np.tril_indices(D, -1)
    rt_all = np.zeros((KPAD, 128, NJ * D), np.float32)
    # strict lower of R.T (a=rows, b=cols) holds U
    rt_all[:K, rows % 128, (rows // 128) * D + cols] = U
    # strict upper of R.T (a=cols, b=rows) holds L
    rt_all[:K, cols % 128, (cols // 128) * D + rows] = L
    dia = np.arange(D)
    rt_all[:K, dia % 128, (dia // 128) * D + dia] = D_diag

    def pack_cols(mat):  # [768, n] -> [128, NJ*n] in (p, (jc, n)) layout
        n = mat.shape[1]
        return (mat.reshape(NJ, 128, n).transpose(1, 0, 2)
                .reshape(128, NJ * n).astype(store_np))

    oodt_h = pack_cols(ood.T.astype(np.float32))

    delta_pad = np.zeros(KPAD, np.float32)
    delta_pad[:K] = delta
    cent_pad = np.zeros((KPAD, D), np.float32)
    cent_pad[:K] = centroids

    in_maps = []
    for mcore in range(NCORES):
        k0 = mcore * KPC
        lab_lo, lab_hi = k0, k0 + KPC
        sel = np.where((labels >= lab_lo) & (labels < lab_hi))[0]
        sel = sel[np.argsort(labels[sel], kind="stable")]
        n_pos = len(sel)
        assert n_pos <= 128, f"core {mcore} has {n_pos} positive samples"

        xpos = np.zeros((D, 128), np.float32)
        slot_label = np.full(128, -1, np.int64)
        if n_pos:
            xpos[:, :n_pos] = (pooled_output[sel] - centroids[labels[sel]]).T
            slot_label[:n_pos] = labels[sel]

        dk_t = np.zeros((128, NCOL), np.float32)
        om_t = np.zeros((128, NCOL), np.float32)
        pm_t = np.zeros((128, NCOL), np.float32)
        for kl in range(KPC):
            kg = k0 + kl
            dk_t[:, kl * NBC:(kl + 1) * NBC] = delta_pad[kg]
            if kg < K:
                om_t[:, kl * NBC:kl * NBC + 4] = 1.0
                pm_t[:, kl * NBC + 4] = (slot_label == kg).astype(np.float32)

        ck_t = (cent_pad[k0:k0 + KPC].reshape(KPC, NJ, 128)
                .transpose(2, 0, 1).reshape(128, KPC * NJ).astype(store_np))

        rt_m = rt_all[k0:k0 + KPC]
        if rt_m.dtype != store_np:
            rt_m = rt_m.astype(store_np)
        in_maps.append({
            "rt": rt_m,
            "oodt": oodt_h,
            "ck": ck_t,
            "xpost": pack_cols(xpos),
            "dk": dk_t,
            "oodmask": om_t,
            "posmask": pm_t,
        })
    return in_maps


def combine(results):
    """Host-side reduction of per-core [128, 4] partials to the 5 outputs."""
    tot = np.zeros(4, np.float64)
    for r in results:
        tot += np.asarray(r["partials"], np.float64).sum(axis=0)
    neg_sum, pos_sum, pos_num, neg_num = tot
    pos_mean = pos_sum / B
    neg_mean = neg_sum / B
    return (np.float32(pos_mean), np.float32(neg_mean),
            np.float32(pos_num), np.float32(neg_num),
            np.float32(pos_mean + neg_mean))


# ======================================================================
# fp8 DoubleRow redesign ("orientation B")
# ----------------------------------------------------------------------
# Row-scale each label's rotate matrix: M_k = diag(1/D_k) R_k, so M has a
# unit diagonal that is EXACT in fp8 after the x32 scale (32 = 2^5).  The
# device computes ps[i,b] = 32*(M_k^T(chunk) @ fp8(samples)) with fp8
# DoubleRow matmuls (contraction 256/instr, 2x bf16 flops), then
#   y = D_i*(ps/32) - z_i,  z_k = R_k c_k  (host, exact)
# via the per-partition scale/bias slots of the Square instruction:
#   sq = (ps * scaleAP + biasAP)^2,  scaleAP = D/32, biasAP = -z.
# A one-hot bf16 matmul reduces sq over the 768 partitions into a
# [19, 640] dist^2 accumulator (640 = 512 ood cols + 128 pos slots).
# The fp8 sample-quantization bias Sum_i D_i^2 r_i^2 is corrected with a
# host-computed [19, 640] tile.  Squares are split ScalarE/DVE (custom
# single-uop affine-square ops); the 6-block sums run on GpSimd.
# ======================================================================

NJP = 3          # contraction pairs (6 chunks of 128 -> 3 DoubleRow pairs)
NIB = 6          # output row blocks (768 = 6*128)
BCOL = B + 128   # 640 moving columns: 512 ood + 128 pos slots
FP8 = mybir.dt.float8e4
BF16 = mybir.dt.bfloat16
DRMODE = mybir.MatmulPerfMode.DoubleRow

_custom_ops = {}


def _register_custom_ops():
    """Register single-uop affine-square DVE ops (idempotent)."""
    if _custom_ops:
        return _custom_ops
    from concourse import dve_ops
    from concourse.dve_spec import C0, C1, Spec, Src0, Src1, lower, sq
    from concourse.dve_spec import _has_src1
    from concourse.dve_uop import DveOpSpec

    existing = {op.name: op for op in dve_ops.OPS}

    def mk(name, spec):
        if name in existing:
            return existing[name]
        row = dve_ops._CUSTOM_DVE_ROW_BASE + len(dve_ops.OPS)
        assert row < 0x20
        shas = {}
        for ver in ("v3", "v4"):
            s = DveOpSpec(name=name, opcode=row, uops=lower(spec, ver=ver),
                          rd1_en=_has_src1(spec))
            shas[ver] = s.sha(ver)
        op = dve_ops.DveOp(name, spec, subdim=False, uops_sha=shas)
        dve_ops.OPS.append(op)
        dve_ops.CUSTOM_DVE_SPECS[name] = spec
        dve_ops._SUB_OPCODE_FOR_NAME[name] = row
        return op

    aff_sq = mk("AFFINE_SQ_ANT", Spec(
        body=sq(Src0 * C0 + C1),
        reference=lambda in0, in1, s0, s1, imm2:
            ((in0.astype(np.float32) * s0 + s1) ** 2).astype(np.float32)))
    aff_sq_add = mk("AFFINE_SQ_ADD_ANT", Spec(
        body=sq(Src0 * C0 + C1) + Src1,
        reference=lambda in0, in1, s0, s1, imm2:
            ((in0.astype(np.float32) * s0 + s1) ** 2 + in1).astype(np.float32)))
    _custom_ops["sq"] = aff_sq
    _custom_ops["sq_add"] = aff_sq_add
    return _custom_ops


def build_program_fp8(warmup=16, debug_acc=False, use_custom_dve=True):
    ops = _register_custom_ops() if use_custom_dve else None
    nc = bacc.Bacc("TRN2", target_bir_lowering=False, debug=False,
                   num_devices=NCORES)
    NCOLS = KPC * NIB  # 114 scale/bias columns

    rt8 = [nc.dram_tensor(f"rt8_{k}", [128, NJP, 2, D], FP8,
                          kind="ExternalInput").ap() for k in range(KPC)]
    ood8 = nc.dram_tensor("ood8", [128, NJP, 2, BCOL], FP8,
                          kind="ExternalInput").ap()
    scale_t = nc.dram_tensor("scale_t", [128, NCOLS], F32,
                             kind="ExternalInput").ap()
    bias_t = nc.dram_tensor("bias_t", [128, NCOLS], F32,
                            kind="ExternalInput").ap()
    ohall = nc.dram_tensor("ohall", [128, KPC, KPC], BF16,
                           kind="ExternalInput").ap()
    dk = nc.dram_tensor("dk", [KPC, 1], F32, kind="ExternalInput").ap()
    om = nc.dram_tensor("om", [KPC, BCOL], F32, kind="ExternalInput").ap()
    pm = nc.dram_tensor("pm", [KPC, BCOL], F32, kind="ExternalInput").ap()
    corr = nc.dram_tensor("corr", [KPC, BCOL], F32, kind="ExternalInput").ap()
    partials = nc.dram_tensor("partials", [KPC, 4], F32,
                              kind="ExternalOutput").ap()
    accdump = None
    if debug_acc:
        accdump = nc.dram_tensor("accdump", [KPC, BCOL], F32,
                                 kind="ExternalOutput").ap()

    LNBETA = float(math.log(BETA))
    const_t = nc.alloc_sbuf_tensor("const-lnbeta", [128, 1], F32)
    nc.gpsimd.memset(const_t.ap(), LNBETA)
    nc.const_aps.aps[(F32, LNBETA)] = const_t.ap()
    nc.all_engine_barrier()

    AF = mybir.ActivationFunctionType
    ALU = mybir.AluOpType

    with tile.TileContext(nc) as tc:
        with (
            tc.tile_pool(name="consts", bufs=1) as cpool,
            tc.tile_pool(name="rtp", bufs=4) as rtpool,
            tc.tile_pool(name="sqp", bufs=8) as sqpool,
            tc.tile_pool(name="addp", bufs=6) as addpool,
            tc.tile_pool(name="fin", bufs=1) as fpool,
            tc.tile_pool(name="psP", bufs=3, space="PSUM") as ppool,
            tc.tile_pool(name="psA", bufs=1, space="PSUM") as apool,
        ):
            ood8_s = cpool.tile([128, NJP, 2, BCOL], FP8)
            nc.sync.dma_start(out=ood8_s[:], in_=ood8[:])
            scale_s = cpool.tile([128, NCOLS], F32)
            nc.sync.dma_start(out=scale_s[:], in_=scale_t[:])
            bias_s = cpool.tile([128, NCOLS], F32)
            nc.sync.dma_start(out=bias_s[:], in_=bias_t[:])
            oh_s = cpool.tile([128, KPC, KPC], BF16)
            nc.sync.dma_start(out=oh_s[:], in_=ohall[:])
            dk_s = cpool.tile([KPC, 1], F32)
            om_s = cpool.tile([KPC, BCOL], F32)
            pm_s = cpool.tile([KPC, BCOL], F32)
            corr_s = cpool.tile([KPC, BCOL], F32)

            # PE clock-ramp warmup: DR matmuls on the ood8 tile.
            for w in range(warmup):
                wps = ppool.tile([128, 1024], F32, tag="P")
                nc.tensor.matmul(wps[:, 0:512], ood8_s[:, 0, :, 0:128],
                                 ood8_s[:, 0, :, 0:512], start=True,
                                 stop=True, perf_mode=DRMODE)

            acc19 = apool.tile([KPC, 1024], F32, tag="acc")

            for k in range(KPC):
                rtk = rtpool.tile([128, NJP, 2, D], FP8)
                for jp in range(NJP):
                    nc.sync.dma_start(out=rtk[:, jp], in_=rt8[k][:, jp])
                sq_tiles = []
                chain = None
                for ib in range(NIB):
                    P = ppool.tile([128, 1024], F32, tag="P")
                    for jp in range(NJP):
                        lhsT = rtk[:, jp, :, ib * 128:(ib + 1) * 128]
                        nc.tensor.matmul(P[:, 0:512], lhsT,
                                         ood8_s[:, jp, :, 0:512],
                                         start=(jp == 0), stop=(jp == NJP - 1),
                                         perf_mode=DRMODE)
                        nc.tensor.matmul(P[:, 512:640], lhsT,
                                         ood8_s[:, jp, :, 512:BCOL],
                                         start=(jp == 0), stop=(jp == NJP - 1),
                                         perf_mode=DRMODE)
                    col = k * NIB + ib
                    sc = scale_s[:, col:col + 1]
                    bi = bias_s[:, col:col + 1]
                    if (not use_custom_dve) or ib < 3:
                        sqt = sqpool.tile([128, BCOL], BF16, tag="sq")
                        nc.scalar.activation(sqt[:], P[:, 0:BCOL], AF.Square,
                                             bias=bi, scale=sc)
                        sq_tiles.append(sqt)
                    elif ib == 3:
                        chain = sqpool.tile([128, BCOL], BF16, tag="dv")
                        nc.vector._custom_dve(ops["sq"], out=chain[:],
                                              in0=P[:, 0:BCOL], s0=sc, s1=bi)
                    else:
                        nchain = sqpool.tile([128, BCOL], BF16, tag="dv")
                        nc.vector._custom_dve(ops["sq_add"], out=nchain[:],
                                              in0=P[:, 0:BCOL], in1=chain[:],
                                              s0=sc, s1=bi)
                        chain = nchain
                # GpSimd combines
                if use_custom_dve:
                    a01 = addpool.tile([128, BCOL], BF16, tag="a")
                    nc.gpsimd.tensor_tensor(out=a01[:], in0=sq_tiles[0][:],
                                            in1=sq_tiles[1][:], op=ALU.add)
                    a012 = addpool.tile([128, BCOL], BF16, tag="a")
                    nc.gpsimd.tensor_tensor(out=a012[:], in0=a01[:],
                                            in1=sq_tiles[2][:], op=ALU.add)
                    sqsum = addpool.tile([128, BCOL], BF16, tag="a")
                    nc.gpsimd.tensor_tensor(out=sqsum[:], in0=a012[:],
                                            in1=chain[:], op=ALU.add)
                else:
                    cur = sq_tiles[0]
                    for nxt in sq_tiles[1:]:
                        dst = addpool.tile([128, BCOL], BF16, tag="a")
                        eng = nc.gpsimd if len(sq_tiles) % 2 else nc.vector
                        eng.tensor_tensor(out=dst[:], in0=cur[:], in1=nxt[:],
                                          op=ALU.add)
                        cur = dst
                    sqsum = cur
                nc.tensor.matmul(acc19[:, 0:512], oh_s[:, k, :],
                                 sqsum[:, 0:512], start=(k == 0),
                                 stop=(k == KPC - 1))
                nc.tensor.matmul(acc19[:, 512:640], oh_s[:, k, :],
                                 sqsum[:, 512:BCOL], start=(k == 0),
                                 stop=(k == KPC - 1))

            nc.sync.dma_start(out=dk_s[:], in_=dk[:])
            nc.sync.dma_start(out=om_s[:], in_=om[:])
            nc.sync.dma_start(out=pm_s[:], in_=pm[:])
            nc.sync.dma_start(out=corr_s[:], in_=corr[:])

            # ---- epilogue on [19, 640] ----
            # corr holds the host-side fp8-residual correction (added):
            # dist^2 = acc + 2*sum(yhat*D*r) + sum(D^2 r^2)
            d2 = fpool.tile([KPC, BCOL], F32)
            nc.vector.tensor_tensor(out=d2[:], in0=acc19[:, 0:BCOL],
                                    in1=corr_s[:], op=ALU.add)
            if debug_acc:
                nc.sync.dma_start(out=accdump[:], in_=d2[:])
            dist = fpool.tile([KPC, BCOL], F32)
            nc.scalar.activation(dist[:], d2[:], AF.Sqrt)
            t = fpool.tile([KPC, BCOL], F32)       # dist - dk
            nc.vector.tensor_scalar_sub(t[:], dist[:], dk_s[:])
            relu_t = fpool.tile([KPC, BCOL], F32)  # (euc - d)+
            nc.scalar.activation(relu_t[:], t[:], AF.Relu)
            nrelu = fpool.tile([KPC, BCOL], F32)   # (d - euc)+
            nc.scalar.activation(nrelu[:], t[:], AF.Relu, scale=-1.0)
            m = fpool.tile([KPC, BCOL], F32)       # 1[d > euc]
            nc.scalar.activation(m[:], nrelu[:], AF.Sign)
            pnum_i = fpool.tile([KPC, BCOL], F32)  # 1[euc > d]
            nc.scalar.activation(pnum_i[:], relu_t[:], AF.Sign)
            e = fpool.tile([KPC, BCOL], F32)       # beta * exp(dk - dist)
            nc.scalar.activation(e[:], t[:], AF.Exp, scale=-1.0, bias=LNBETA)
            nc.vector.tensor_scalar_min(e[:], e[:], BETA)
            inb = fpool.tile([KPC, BCOL], F32)     # dk - dist + beta
            nc.scalar.activation(inb[:], t[:], AF.Copy, scale=-1.0, bias=BETA)
            d1 = fpool.tile([KPC, BCOL], F32)
            nc.vector.tensor_tensor(out=d1[:], in0=inb[:], in1=e[:],
                                    op=ALU.subtract)
            d2b = fpool.tile([KPC, BCOL], F32)
            nc.vector.tensor_tensor(out=d2b[:], in0=m[:], in1=d1[:],
                                    op=ALU.mult)
            pl = fpool.tile([KPC, BCOL], F32)
            nc.vector.tensor_tensor(out=pl[:], in0=e[:], in1=d2b[:],
                                    op=ALU.add)

            out4 = fpool.tile([KPC, 4], F32)
            for idx, (a, b_) in enumerate([(pl, om_s), (relu_t, pm_s),
                                           (pnum_i, pm_s), (m, pm_s)]):
                tmp = fpool.tile([KPC, BCOL], F32, tag="redtmp")
                nc.vector.tensor_tensor(out=tmp[:], in0=a[:], in1=b_[:],
                                        op=ALU.mult)
                nc.vector.tensor_reduce(out=out4[:, idx:idx + 1], in_=tmp[:],
                                        axis=mybir.AxisListType.X, op=ALU.add)
            nc.sync.dma_start(out=partials[:], in_=out4[:])

    nc.compile()
    return nc


def prep_inputs_fp8(pooled_output, centroids, delta, L, U, D_diag, ood,
                    labels):
    import ml_dtypes
    f8 = ml_dtypes.float8_e4m3
    bf = ml_dtypes.bfloat16

    pooled_output = np.asarray(pooled_output, np.float32)
    centroids = np.asarray(centroids, np.float32)
    delta = np.asarray(delta, np.float32)
    L = np.asarray(L, np.float32)
    U = np.asarray(U, np.float32)
    D_diag = np.asarray(D_diag, np.float32)
    ood = np.asarray(ood, np.float32)
    labels = np.asarray(labels).astype(np.int64)

    # R^T in packed layout [k, p, jc, i]: value = R[i, j], j = jc*128+p.
    rows, cols = np.tril_indices(D, -1)
    rt_all = np.zeros((KPAD, 128, NJ, D), np.float32)
    rt_flat = rt_all.reshape(KPAD, 128, NJ * D)
    rt_flat[:K, rows % 128, (rows // 128) * D + cols] = U
    rt_flat[:K, cols % 128, (cols // 128) * D + rows] = L
    dia = np.arange(D)
    rt_flat[:K, dia % 128, (dia // 128) * D + dia] = D_diag

    delta_pad = np.zeros(KPAD, np.float32)
    delta_pad[:K] = delta
    cent_pad = np.zeros((KPAD, D), np.float32)
    cent_pad[:K] = centroids
    D_pad = np.ones((KPAD, D), np.float32)
    D_pad[:K] = D_diag

    # z_k = R_k c_k from the packed layout (before row scaling)
    cre = cent_pad.reshape(KPAD, NJ, 128).transpose(0, 2, 1)  # [k, p, jc]
    z_all = np.einsum('kpji,kpj->ki', rt_all, cre, optimize=True)

    # row-scale (divide by D over the i axis), x32, fp8
    rt_all /= D_pad[:, None, None, :]
    rt8_all = (rt_all * 32.0).astype(f8)      # [KPAD, 128, 6, 768]
    rt8_all = rt8_all.reshape(KPAD, 128, NJP, 2, D)
    del rt_all

    # fp8 samples + quantization residual
    ood8_cols = ood.astype(f8)                 # [512, 768]
    r_ood = ood - ood8_cols.astype(np.float32)
    pooled8 = pooled_output.astype(f8)
    r_pooled = pooled_output - pooled8.astype(np.float32)

    def pack_cols(colmat8):
        # [n, 768] fp8 -> [128, NJP, 2, n]
        n = colmat8.shape[0]
        return (colmat8.reshape(n, NJP, 2, 128)
                .transpose(3, 1, 2, 0).copy())

    in_maps = []
    for mcore in range(NCORES):
        k0 = mcore * KPC
        sel = np.where((labels >= k0) & (labels < k0 + KPC))[0]
        n_pos = len(sel)
        assert n_pos <= 128, f"core {mcore} has {n_pos} positive samples"

        samp8 = np.zeros((BCOL, D), f8)
        samp8[0:B] = ood8_cols
        samp8[B:B + n_pos] = pooled8[sel]
        ood8_t = pack_cols(samp8)

        r_all = np.zeros((BCOL, D), np.float32)
        r_all[0:B] = r_ood
        r_all[B:B + n_pos] = r_pooled[sel]

        NCOLS = KPC * NIB
        scale_h = np.zeros((128, NCOLS), np.float32)
        bias_h = np.zeros((128, NCOLS), np.float32)
        om_h = np.zeros((KPC, BCOL), np.float32)
        pm_h = np.zeros((KPC, BCOL), np.float32)
        dk_h = np.zeros((KPC, 1), np.float32)
        corr_h = np.zeros((KPC, BCOL), np.float32)
        r2 = r_all ** 2                       # [640, 768]
        for kl in range(KPC):
            kg = k0 + kl
            Dk = D_pad[kg]
            zk = z_all[kg]
            for ib in range(NIB):
                col = kl * NIB + ib
                scale_h[:, col] = Dk[ib * 128:(ib + 1) * 128] / 32.0
                bias_h[:, col] = -zk[ib * 128:(ib + 1) * 128]
            dk_h[kl, 0] = delta_pad[kg]
            corr_h[kl] = r2 @ (Dk * Dk)
            if kg < K:
                om_h[kl, 0:B] = 1.0
                if n_pos:
                    pm_h[kl, B:B + n_pos] = (labels[sel] == kg)

        oh_h = np.zeros((128, KPC, KPC), bf)
        for kl in range(KPC):
            oh_h[:, kl, kl] = 1.0

        im = {f"rt8_{kl}": rt8_all[k0 + kl] for kl in range(KPC)}
        im.update({
            "ood8": ood8_t, "scale_t": scale_h, "bias_t": bias_h,
            "ohall": oh_h, "dk": dk_h, "om": om_h, "pm": pm_h,
            "corr": corr_h,
        })
        in_maps.append(im)
    return in_maps


def combine_fp8(results):
    tot = np.zeros(4, np.float64)
    for r in results:
        tot += np.asarray(r["partials"], np.float64).sum(axis=0)
    neg_sum, pos_sum, pos_num, neg_num = tot
    pos_mean = pos_sum / B
    neg_mean = neg_sum / B
    return (np.float32(pos_mean), np.float32(neg_mean),
            np.float32(pos_num), np.float32(neg_num),
            np.float32(pos_mean + neg_mean))


# ======================================================================
# fp8 v2 — pos-slot trim, deferred reduce, transposed epilogue
# ----------------------------------------------------------------------
# Same orientation-B math as fp8 above, plus:
#   * pos term uses 16 per-label slots (max 11 samples share a label)
#     instead of 128 shared slots -> 528 moving cols instead of 640.
#   * HAM warmup depends only on the first third of the ood8 load.
#   * each label's one-hot partition-reduce matmul is emitted after the
#     NEXT label's ib3 matmuls, hiding the square/add chain latency.
#   * epilogue: dist^2 [19,528] is PE-transposed into [128,95] so the
#     elementwise tail runs on all 128 lanes; sqrt = exp(0.5 ln x) and
#     where(in, d-dist+B, B e^{d-dist}) = B e^{-relu(t)} + relu(-t)
#     keep every activation inside natural_log_exp_and_others (one
#     table load, hoisted to the head by dummy Ln/Exp activations).
# ======================================================================

NPOS = 16            # pos slots per label
BC2 = B + NPOS       # 528 moving cols
BMOV = B + KPC * NPOS  # 816: ood cols + all labels' pos slots, one tensor
NCOLT = 5 * KPC      # 95 transposed epilogue cols
TRO = [0, 128, 256, 384, 400]   # transpose block offsets into [19, 528]


def _dedup_ldweights(nc):
    """Drop InstLdweights that reload the exact weights already resident.

    The tile legalizer emits one LDW per matmul; our 512-col/16-col pair
    shares a stationary, so every second LDW is redundant and its ~107ns
    load blocks the next pair's prefetch (measured +50ns/pair). Weights
    persist in the PE until the next LDW, so dropping the duplicate is
    semantics-preserving. Skips LDWs carrying sync_info; any matmul with
    is_transpose (self-loading) invalidates the resident-weights state.
    """
    removed = 0
    for blk in nc.main_func.blocks:
        insts = blk.instructions
        last_key = None
        drop = []
        for idx, inst in enumerate(insts):
            if isinstance(inst, mybir.InstLdweights):
                si = inst.sync_info
                has_sync = si is not None and (len(si.on_wait) > 0 or
                                               len(si.on_update) > 0)
                key = (inst.concise(), )
                if key == last_key and not has_sync:
                    drop.append(idx)
                else:
                    last_key = key
            elif isinstance(inst, mybir.InstMatmult):
                if inst.is_transpose:
                    last_key = None
        for idx in reversed(drop):
            del insts[idx]
        removed += len(drop)
    return removed


def build_program_fp8_v2(warmup=10, debug_acc=False):
    ops = _register_custom_ops()
    nc = bacc.Bacc("TRN2", target_bir_lowering=False, debug=False,
                   num_devices=NCORES)
    NCOLS = KPC * NIB  # 114 scale/bias columns
    CW = 2 * NCOLS + 3 * NCOLT  # packed f32 consts: scale|bias|dkt|om|pm

    rt8 = [nc.dram_tensor(f"rt8_{k}", [128, NJP, 2, D], FP8,
                          kind="ExternalInput").ap() for k in range(KPC)]
    mov8 = nc.dram_tensor("mov8", [128, NJP, 2, BMOV], FP8,
                          kind="ExternalInput").ap()
    cst = nc.dram_tensor("cst", [128, CW], F32, kind="ExternalInput").ap()
    ohs = nc.dram_tensor("ohs", [128, 2 * KPC - 1], BF16,
                         kind="ExternalInput").ap()
    ident = nc.dram_tensor("ident", [KPC, KPC], F32,
                           kind="ExternalInput").ap()
    corr = nc.dram_tensor("corr", [KPC, BC2], F32,
                          kind="ExternalInput").ap()
    partials = nc.dram_tensor("partials", [128, 4], F32,
                              kind="ExternalOutput").ap()
    accdump = None
    if debug_acc:
        accdump = nc.dram_tensor("accdump", [128, NCOLT], F32,
                                 kind="ExternalOutput").ap()

    LNBETA = float(math.log(BETA))
    const_t = nc.alloc_sbuf_tensor("const-lnbeta", [128, 1], F32)
    nc.gpsimd.memset(const_t.ap(), LNBETA)
    nc.const_aps.aps[(F32, LNBETA)] = const_t.ap()
    nc.all_engine_barrier()

    AF = mybir.ActivationFunctionType
    ALU = mybir.AluOpType

    with tile.TileContext(nc) as tc:
        with (
            tc.tile_pool(name="consts", bufs=1) as cpool,
            tc.tile_pool(name="rtp", bufs=4) as rtpool,
            tc.tile_pool(name="sqp", bufs=8) as sqpool,
            tc.tile_pool(name="addp", bufs=4) as addpool,
            tc.tile_pool(name="fin", bufs=1) as fpool,
            tc.tile_pool(name="psP", bufs=3, space="PSUM") as ppool,
            tc.tile_pool(name="psA", bufs=1, space="PSUM") as apool,
        ):
            # dummy Ln pins the act table to a square+ln set so the main
            # loop and the epilogue Ln run with zero table switches; the
            # single switch to the exp set happens once, in the tail.
            dumm = fpool.tile([128, 1], F32, tag="dummy")
            nc.scalar.activation(dumm[:], const_t.ap(), AF.Ln)

            mov8_s = cpool.tile([128, NJP, 2, BMOV], FP8)
            nc.sync.dma_start(out=mov8_s[:, 0], in_=mov8[:, 0])
            rt0 = rtpool.tile([128, NJP, 2, D], FP8)
            for jp in range(NJP):
                nc.sync.dma_start(out=rt0[:, jp], in_=rt8[0][:, jp])
            for jp in range(1, NJP):
                nc.sync.dma_start(out=mov8_s[:, jp], in_=mov8[:, jp])
            cst_s = cpool.tile([128, CW], F32)
            nc.sync.dma_start(out=cst_s[:], in_=cst[:])
            ohs_s = cpool.tile([128, 2 * KPC - 1], BF16)
            nc.sync.dma_start(out=ohs_s[:], in_=ohs[:])
            ident_s = cpool.tile([KPC, KPC], F32)
            nc.sync.dma_start(out=ident_s[:], in_=ident[:])
            corr_s = cpool.tile([KPC, BC2], F32)
            nc.sync.dma_start(out=corr_s[:], in_=corr[:])
            scale_s = cst_s[:, 0:NCOLS]
            bias_s = cst_s[:, NCOLS:2 * NCOLS]
            dkt_s = cst_s[:, 2 * NCOLS:2 * NCOLS + NCOLT]
            om_s = cst_s[:, 2 * NCOLS + NCOLT:2 * NCOLS + 2 * NCOLT]
            pm_s = cst_s[:, 2 * NCOLS + 2 * NCOLT:CW]

            # HAM warmup: DR matmuls gated only on mov8 chunk 0.
            for w in range(warmup):
                wps = ppool.tile([128, 1024], F32, tag="ps")
                nc.tensor.matmul(wps[:, 0:512], mov8_s[:, 0, :, 0:128],
                                 mov8_s[:, 0, :, 0:512], start=True,
                                 stop=True, perf_mode=DRMODE)

            A = apool.tile([128, 1024], F32, tag="acc")

            def emit_oh(kk, sqsum):
                nc.tensor.matmul(A[0:KPC, 0:512],
                                 ohs_s[:, KPC - 1 - kk:2 * KPC - 1 - kk],
                                 sqsum[:, 0:512], start=(kk == 0),
                                 stop=False)
                nc.tensor.matmul(A[0:KPC, 512:512 + NPOS],
                                 ohs_s[:, KPC - 1 - kk:2 * KPC - 1 - kk],
                                 sqsum[:, 512:BC2], start=(kk == 0),
                                 stop=(kk == KPC - 1))

            pend = None
            for k in range(KPC):
                if k == 0:
                    rtk = rt0
                else:
                    rtk = rtpool.tile([128, NJP, 2, D], FP8)
                    nc.sync.dma_start(out=rtk[:], in_=rt8[k][:])
                sq01 = []
                chain = None
                for ib in range(NIB):
                    P = ppool.tile([128, 1024], F32, tag="ps")
                    for jp in range(NJP):
                        lhsT = rtk[:, jp, :, ib * 128:(ib + 1) * 128]
                        nc.tensor.matmul(P[:, 0:512], lhsT,
                                         mov8_s[:, jp, :, 0:B],
                                         start=(jp == 0), stop=(jp == NJP - 1),
                                         perf_mode=DRMODE)
                        nc.tensor.matmul(
                            P[:, 512:512 + NPOS], lhsT,
                            mov8_s[:, jp, :, B + k * NPOS:B + (k + 1) * NPOS],
                            start=(jp == 0), stop=(jp == NJP - 1),
                            perf_mode=DRMODE)
                    if ib == 3 and pend is not None:
                        emit_oh(*pend)
                        pend = None
                    col = k * NIB + ib
                    sc = scale_s[:, col:col + 1]
                    bi = bias_s[:, col:col + 1]
                    if ib < 2:
                        sqt = sqpool.tile([128, BC2], BF16, tag="sq")
                        nc.scalar.activation(sqt[:], P[:, 0:BC2], AF.Square,
                                             bias=bi, scale=sc)
                        sq01.append(sqt)
                    elif ib == 2:
                        chain = sqpool.tile([128, BC2], BF16, tag="dv")
                        nc.vector._custom_dve(ops["sq"], out=chain[:],
                                              in0=P[:, 0:BC2], s0=sc, s1=bi)
                    else:
                        nchain = sqpool.tile([128, BC2], BF16, tag="dv")
                        nc.vector._custom_dve(ops["sq_add"], out=nchain[:],
                                              in0=P[:, 0:BC2], in1=chain[:],
                                              s0=sc, s1=bi)
                        chain = nchain
                g1 = addpool.tile([128, BC2], BF16, tag="a")
                nc.gpsimd.tensor_tensor(out=g1[:], in0=sq01[0][:],
                                        in1=sq01[1][:], op=ALU.add)
                sqsum = addpool.tile([128, BC2], BF16, tag="a")
                nc.gpsimd.tensor_tensor(out=sqsum[:], in0=g1[:],
                                        in1=chain[:], op=ALU.add)
                pend = (k, sqsum)
            emit_oh(*pend)

            # ---- epilogue ----
            d2s = fpool.tile([KPC, BC2], F32)   # dist^2 + fp8 correction
            nc.vector.tensor_tensor(out=d2s[:], in0=A[0:KPC, 0:BC2],
                                    in1=corr_s[:], op=ALU.add)
            # PE transpose [19, 528] -> [128, 95] in A cols 640:735
            for c, off in enumerate(TRO):
                nc.tensor.transpose(
                    A[:, 640 + c * KPC:640 + (c + 1) * KPC],
                    d2s[:, off:off + 128], ident_s[:])
            tr = A[:, 640:640 + NCOLT]
            if debug_acc:
                nc.sync.dma_start(out=accdump[:], in_=tr)
            lnv = fpool.tile([128, NCOLT], F32)
            nc.scalar.activation(lnv[:], tr, AF.Ln)
            dist = fpool.tile([128, NCOLT], F32)   # exp(0.5 ln d2)
            nc.scalar.activation(dist[:], lnv[:], AF.Exp, scale=0.5)
            t = fpool.tile([128, NCOLT], F32)      # dist - dk
            nc.vector.tensor_tensor(out=t[:], in0=dist[:], in1=dkt_s,
                                    op=ALU.subtract)
            relu_t = fpool.tile([128, NCOLT], F32)  # (euc - d)+
            nc.scalar.activation(relu_t[:], t[:], AF.Relu)
            nrelu = fpool.tile([128, NCOLT], F32)   # (d - euc)+
            nc.scalar.activation(nrelu[:], t[:], AF.Relu, scale=-1.0)
            e2 = fpool.tile([128, NCOLT], F32)      # beta * exp(-relu_t)
            nc.scalar.activation(e2[:], relu_t[:], AF.Exp, scale=-1.0,
                                 bias=LNBETA)
            pl = fpool.tile([128, NCOLT], F32)      # == where(d>dist, ...)
            nc.vector.tensor_tensor(out=pl[:], in0=e2[:], in1=nrelu[:],
                                    op=ALU.add)
            pnum_i = fpool.tile([128, NCOLT], F32)  # 1[euc > d]
            nc.scalar.activation(pnum_i[:], relu_t[:], AF.Sign)
            m = fpool.tile([128, NCOLT], F32)       # 1[euc < d]
            nc.scalar.activation(m[:], nrelu[:], AF.Sign)

            out4 = fpool.tile([128, 4], F32)
            tmp4 = fpool.tile([128, 4, NCOLT], F32)
            for idx, (a, b_) in enumerate([(pl, om_s), (relu_t, pm_s),
                                           (pnum_i, pm_s), (m, pm_s)]):
                nc.vector.tensor_tensor(out=tmp4[:, idx, :], in0=a[:],
                                        in1=b_, op=ALU.mult)
            nc.vector.tensor_reduce(out=out4[:], in_=tmp4[:],
                                    axis=mybir.AxisListType.X, op=ALU.add)
            nc.sync.dma_start(out=partials[:], in_=out4[:])

    _dedup_ldweights(nc)
    nc.compile()
    return nc


def prep_inputs_fp8_v2(pooled_output, centroids, delta, L, U, D_diag, ood,
                       labels):
    import ml_dtypes
    f8 = ml_dtypes.float8_e4m3
    bf = ml_dtypes.bfloat16

    pooled_output = np.asarray(pooled_output, np.float32)
    centroids = np.asarray(centroids, np.float32)
    delta = np.asarray(delta, np.float32)
    L = np.asarray(L, np.float32)
    U = np.asarray(U, np.float32)
    D_diag = np.asarray(D_diag, np.float32)
    ood = np.asarray(ood, np.float32)
    labels = np.asarray(labels).astype(np.int64)

    rows, cols = np.tril_indices(D, -1)
    rt_all = np.zeros((KPAD, 128, NJ, D), np.float32)
    rt_flat = rt_all.reshape(KPAD, 128, NJ * D)
    rt_flat[:K, rows % 128, (rows // 128) * D + cols] = U
    rt_flat[:K, cols % 128, (cols // 128) * D + rows] = L
    dia = np.arange(D)
    rt_flat[:K, dia % 128, (dia // 128) * D + dia] = D_diag

    delta_pad = np.zeros(KPAD, np.float32)
    delta_pad[:K] = delta
    cent_pad = np.zeros((KPAD, D), np.float32)
    cent_pad[:K] = centroids
    D_pad = np.ones((KPAD, D), np.float32)
    D_pad[:K] = D_diag

    cre = cent_pad.reshape(KPAD, NJ, 128).transpose(0, 2, 1)  # [k, p, jc]
    z_all = np.einsum('kpji,kpj->ki', rt_all, cre, optimize=True)

    rt_all /= D_pad[:, None, None, :]
    rt8_all = (rt_all * 32.0).astype(f8)
    rt8_all = rt8_all.reshape(KPAD, 128, NJP, 2, D)
    del rt_all

    ood8_cols = ood.astype(f8)
    r_ood = ood - ood8_cols.astype(np.float32)
    pooled8 = pooled_output.astype(f8)
    r_pooled = pooled_output - pooled8.astype(np.float32)
    r2_ood = r_ood ** 2
    r2_pooled = r_pooled ** 2

    def pack_cols(colmat8):
        # [n, 768] fp8 -> [128, NJP, 2, n]
        n = colmat8.shape[0]
        return (colmat8.reshape(n, NJP, 2, 128)
                .transpose(3, 1, 2, 0).copy())

    NCOLS = KPC * NIB
    ident_h = np.eye(KPC, dtype=np.float32)
    ohs_h = np.zeros((128, 2 * KPC - 1), bf)
    ohs_h[:, KPC - 1] = 1.0

    in_maps = []
    for mcore in range(NCORES):
        k0 = mcore * KPC
        sel = np.where((labels >= k0) & (labels < k0 + KPC))[0]

        pos_cols8 = np.zeros((KPC * NPOS, D), f8)
        scale_h = np.zeros((128, NCOLS), np.float32)
        bias_h = np.zeros((128, NCOLS), np.float32)
        corr_h = np.zeros((KPC, BC2), np.float32)
        dk_h = np.zeros((128, NCOLT), np.float32)
        om_h = np.zeros((128, NCOLT), np.float32)
        pm_h = np.zeros((128, NCOLT), np.float32)

        for kl in range(KPC):
            kg = k0 + kl
            Dk = D_pad[kg]
            zk = z_all[kg]
            for ib in range(NIB):
                col = kl * NIB + ib
                scale_h[:, col] = Dk[ib * 128:(ib + 1) * 128] / 32.0
                bias_h[:, col] = -zk[ib * 128:(ib + 1) * 128]
            corr_h[kl, 0:B] = r2_ood @ (Dk * Dk)
            # padded pos slots get d2 >= 1 so Ln never sees 0 (NaN/-inf
            # would poison the masked reduces via 0*NaN).
            corr_h[kl, B:] = 1.0
            ksel = sel[labels[sel] == kg]
            n_k = len(ksel)
            assert n_k <= NPOS, f"label {kg} has {n_k} > {NPOS} samples"
            if n_k:
                pos_cols8[kl * NPOS:kl * NPOS + n_k] = pooled8[ksel]
                corr_h[kl, B:B + n_k] = r2_pooled[ksel] @ (Dk * Dk)
            # transposed-layout consts: col = c*19 + kl
            for c in range(5):
                tc_ = c * KPC + kl
                dk_h[:, tc_] = delta_pad[kg]
                if kg < K:
                    if c < 4:
                        om_h[:, tc_] = 1.0
                    else:
                        # rows p>=112 are pos slots s = p-112
                        pm_h[112:112 + n_k, tc_] = 1.0

        mov_cols8 = np.concatenate([ood8_cols, pos_cols8], axis=0)
        cst_h = np.concatenate([scale_h, bias_h, dk_h, om_h, pm_h], axis=1)
        im = {f"rt8_{kl}": rt8_all[k0 + kl] for kl in range(KPC)}
        im.update({
            "mov8": pack_cols(mov_cols8), "cst": cst_h, "ohs": ohs_h,
            "ident": ident_h, "corr": corr_h,
        })
        in_maps.append(im)
    return in_maps


def kernel(pooled_output, centroids, delta, L, U, D_diag, ood, labels,
           mode="fp8v2", trace=False, debug_acc=False):
    if mode == "fp8v2":
        labels_arr = np.asarray(labels).astype(np.int64)
        counts = np.bincount(labels_arr, minlength=K)
        if counts.max() > NPOS:
            mode = "fp8"  # fallback: per-label slots overflow
    if mode == "fp8v2":
        key = ("fp8v2", debug_acc)
        if key not in _prog_cache:
            _prog_cache[key] = build_program_fp8_v2(debug_acc=debug_acc)
        nc = _prog_cache[key]
        in_maps = prep_inputs_fp8_v2(pooled_output, centroids, delta, L, U,
                                     D_diag, ood, labels)
        res = run_bass_kernel_spmd(nc, in_maps, list(range(NCORES)),
                                   trace=trace)
        out = combine_fp8(res.results)
        if trace:
            return out, res
        return out
    if mode == "fp8":
        key = ("fp8", debug_acc)
        if key not in _prog_cache:
            _prog_cache[key] = build_program_fp8(debug_acc=debug_acc)
        nc = _prog_cache[key]
        in_maps = prep_inputs_fp8(pooled_output, centroids, delta, L, U,
                                  D_diag, ood, labels)
        res = run_bass_kernel_spmd(nc, in_maps, list(range(NCORES)),
                                   trace=trace)
        out = combine_fp8(res.results)
    else:
        if mode not in _prog_cache:
            _prog_cache[mode] = build_program(mode)
        nc = _prog_cache[mode]
        in_maps = prep_inputs(pooled_output, centroids, delta, L, U, D_diag,
                              ood, labels, mode=mode)
        res = run_bass_kernel_spmd(nc, in_maps, list(range(NCORES)),
                                   trace=trace)
        out = combine(res.results)
    if trace:
        return out, res
    return out

